# revision 60
# baseline (speedup 1.0000x reference)
"""TransformerConv (heads=1) + ELU layer as a Bass/Tile kernel on 8 NeuronCores.

v4 strategy (1D partition by target node, host-staged per-edge features):
  - dst nodes sharded 8 ways (12500/core, padded to 98 blocks x 128).
  - The host performs the halo exchange / sharding: for every edge (grouped
    into 128-edge chunks per dst block, shared chunk-count profile across
    cores) it emits the source-node and dest-node feature rows of x,
    TRANSPOSED into the [d_in, edge] layout the TensorEngine wants for its
    stationary operand, as two flat DRAM streams. No indirect DMA at all -
    random row gathers are latency-bound on the SDMA engines (~19 GB/s
    measured), while these streams run at line rate.
  - Per block the device streams xsT/xdT, computes per-edge q = xd@Wq + bq
    (scale folded), k = xs@Wk (bk cancels in the per-dst softmax),
    v = xs@Wv (bv folds into the skip bias since sum alpha = 1);
    logits = rowsum(q*k); ex = exp(logits) (no max subtraction - logits are
    O(1)); one-hot oh[e,d] = (iota == dstloc) built on GPSIMD; V rows
    scaled by ex with ex appended as column 128; TensorEngine scatter-adds
    [agg | den] += oh^T @ [Vex | ex].
  - skip = x@Ws + (bs+bv) for the dst slice computed once into SBUF.
  - Epilogue per block: out = elu(agg/den + skip), streamed to DRAM.
Pad edge slots have zero features and dstloc=255 so their one-hot columns
are zero - they contribute nothing.
"""
import math
import numpy as np
import ml_dtypes

BF16 = ml_dtypes.bfloat16

N, E, D = 100000, 800000, 128
M_CORES = 8
DPC = N // M_CORES                 # 12500
NB = (DPC + 127) // 128            # 98
DST_PAD = NB * 128                 # 12544
SCALE = 1.0 / math.sqrt(D)
TW = 2048                          # skip-phase row-tile width
CMAX_HW = 8                        # chunks per block (uniform; 2 PSUM banks)


def _host_prep(edge_index):
    """Pack dsts into blocks against a shared per-block chunk-count profile.

    Returns (srcn [M,128,S], dstn [M,128,S], dstloc [M,128,S], perm
    [M,DST_PAD], cc tuple) where S = sum(cc); srcn/dstn are the global node
    ids of each edge slot (-1 for pads); perm[c][device_row] = local dst id
    (or >= DPC for pads)."""
    src = np.asarray(edge_index[0], dtype=np.int64)
    dst = np.asarray(edge_index[1], dtype=np.int64)
    core = dst // DPC
    ld = dst - core * DPC

    # uniform CMAX_HW chunks per block; grow the block count until the
    # per-core LPT packing of dst nodes fits every block under its cap
    nb = NB
    while True:
        dst_pad = nb * 128
        cc = np.full(nb, CMAX_HW, np.int64)
        caps = cc * 128
        deg = np.zeros((M_CORES, dst_pad), np.int64)
        for c in range(M_CORES):
            deg[c, :DPC] = np.bincount(ld[core == c], minlength=DPC)[:DPC]
        assign = np.zeros((M_CORES, dst_pad), np.int64)  # local dst -> block
        ok = True
        for c in range(M_CORES):
            order = np.argsort(-deg[c])
            loads = np.zeros(nb, np.int64)
            for k in range(128):
                batch = order[k * nb:(k + 1) * nb]
                binord = np.argsort(loads - caps)
                assign[c, batch] = binord
                loads[binord] += deg[c, batch]
            if (loads > caps).any():
                ok = False
                break
        if ok:
            break
        nb += 1
        if nb > NB + 16:
            raise RuntimeError("packing failed")
    S = int(cc.sum())
    colbase = np.concatenate([[0], np.cumsum(cc)[:-1]])

    srcn = np.full((M_CORES, 128, S), -1, np.int64)
    dstn = np.full((M_CORES, 128, S), -1, np.int64)
    dstloc = np.full((M_CORES, 128, S), 255.0, np.float32)
    perm = np.zeros((M_CORES, dst_pad), np.int64)
    for c in range(M_CORES):
        blk = assign[c]                      # local dst -> block
        order = np.argsort(blk, kind="stable")
        blk_sorted = blk[order]
        starts = np.searchsorted(blk_sorted, np.arange(nb))
        lane = np.arange(dst_pad) - starts[blk_sorted]
        rows = blk_sorted * 128 + lane
        perm[c, rows] = order                # device row -> local dst
        lane_of = np.zeros(dst_pad, np.int64)
        lane_of[order] = lane

        sel = core == c
        e_ld = ld[sel]
        e_src = src[sel]
        e_blk = blk[e_ld]
        g_order = np.argsort(e_blk, kind="stable")
        gb = e_blk[g_order]
        counts = np.bincount(gb, minlength=nb)
        if (counts > cc * 128).any():
            raise RuntimeError("packing overflow")
        estarts = np.concatenate([[0], np.cumsum(counts)[:-1]])
        j = np.arange(len(gb)) - estarts[gb]
        c_of = j // 128
        p_of = j % 128
        scol = colbase[gb] + c_of
        srcn[c, p_of, scol] = e_src[g_order]
        dstn[c, p_of, scol] = (e_ld[g_order] + c * DPC)
        dstloc[c, p_of, scol] = lane_of[e_ld[g_order]].astype(np.float32)
    return srcn, dstn, dstloc, perm, tuple(int(x) for x in cc)


def _build_nc(cc, dst_pad=None, tw=None):
    from contextlib import ExitStack
    import concourse.tile as tile
    from concourse import bacc, mybir

    if dst_pad is None:
        dst_pad = len(cc) * 128
    if tw is None:
        tw = TW

    fp32 = mybir.dt.float32
    bf16 = mybir.dt.bfloat16
    i32 = mybir.dt.int32
    Alu = mybir.AluOpType
    Act = mybir.ActivationFunctionType

    nc = bacc.Bacc("TRN2", target_bir_lowering=False, debug=False)
    nb = len(cc)
    S = int(sum(cc))
    colbase = [0]
    for x in cc[:-1]:
        colbase.append(colbase[-1] + x)

    xsT = nc.dram_tensor("xsT", [128, S * 128], bf16, kind="ExternalInput").ap()
    xdT = nc.dram_tensor("xdT", [128, S * 128], bf16, kind="ExternalInput").ap()
    xsN = nc.dram_tensor("xsN", [128, S * 128], bf16, kind="ExternalInput").ap()
    xTs = nc.dram_tensor("xTs", [128, dst_pad], bf16, kind="ExternalInput").ap()
    Amat = nc.dram_tensor("Amat", [128, 128], bf16, kind="ExternalInput").ap()
    utld = nc.dram_tensor("utld", [1, CMAX_HW * 128], bf16, kind="ExternalInput").ap()
    Wv = nc.dram_tensor("Wv", [128, 128], bf16, kind="ExternalInput").ap()
    Ws = nc.dram_tensor("Ws", [128, 128], bf16, kind="ExternalInput").ap()
    bsv1 = nc.dram_tensor("bsv1", [1, 128], bf16, kind="ExternalInput").ap()
    oh_d = nc.dram_tensor("ohs", [128, S * 128], bf16, kind="ExternalInput").ap()

    out_d = nc.dram_tensor("out", [dst_pad, 128], fp32, kind="ExternalOutput").ap()

    with tile.TileContext(nc) as tc, ExitStack() as ctx:
        const_p = ctx.enter_context(tc.tile_pool(name="const", bufs=1))

        w_a = const_p.tile([128, 128], bf16, tag="wa")
        u_tl = const_p.tile([1, CMAX_HW * 128], bf16, tag="utl")
        w_v = const_p.tile([128, 128], bf16, tag="wv")
        w_s = const_p.tile([128, 128], bf16, tag="ws")
        b_sv = const_p.tile([1, 128], bf16, tag="bsv")
        nc.sync.dma_start(w_a[:], Amat[:])
        nc.sync.dma_start(u_tl[:], utld[:])
        nc.sync.dma_start(w_v[:], Wv[:])
        nc.sync.dma_start(w_s[:], Ws[:])
        nc.sync.dma_start(b_sv[:], bsv1[:])

        ones1 = const_p.tile([1, 128], bf16, tag="ones1")
        nc.vector.memset(ones1[:], 1.0)

        skip_sb = const_p.tile([128, nb, 128], bf16, tag="skip")

        # ---------------- phase 1: skip = x@Ws + (bs+bv) for dst slice ------
        n_full_b = dst_pad // tw
        tiles1b = [(i * tw, tw) for i in range(n_full_b)]
        if dst_pad % tw:
            tiles1b.append((n_full_b * tw, dst_pad % tw))
        with tc.tile_pool(name="p1x", bufs=3) as p1x, \
             tc.tile_pool(name="p1ps", bufs=4, space="PSUM") as p1ps:
            for (base, w) in tiles1b:
                nj = w // 128
                xt = p1x.tile([128, w], bf16, tag="xst")
                nc.sync.dma_start(xt[:], xTs[:, base:base + w])
                for j in range(nj):
                    lhs = xt[:, j * 128:(j + 1) * 128]
                    blk = base // 128 + j
                    ps = p1ps.tile([128, 128], fp32, tag="ps1")
                    nc.tensor.matmul(out=ps[:], lhsT=lhs, rhs=w_s[:], start=True, stop=False)
                    nc.tensor.matmul(out=ps[:], lhsT=ones1[:], rhs=b_sv[:], start=False, stop=True)
                    nc.scalar.activation(skip_sb[:, blk, :], ps[:], Act.Copy)

        # ---------------- phase 2: edge attention + scatter -----------------
        with tc.tile_pool(name="xs", bufs=5) as xs_p, \
             tc.tile_pool(name="xd", bufs=5) as xd_p, \
             tc.tile_pool(name="ew", bufs=4) as ew_p, \
             tc.tile_pool(name="ohp", bufs=5) as oh_p, \
             tc.tile_pool(name="epi", bufs=4) as epi_p, \
             tc.tile_pool(name="pqp", bufs=2, space="PSUM") as pq_p, \
             tc.tile_pool(name="pvp", bufs=1, space="PSUM") as pv_p, \
             tc.tile_pool(name="aps", bufs=2, space="PSUM") as ag_p:
            for b in range(nb):
                cmax = cc[b]
                cb = colbase[b]
                w = cmax * 128
                xst = xs_p.tile([128, CMAX_HW * 128], bf16, tag="xst")
                nc.sync.dma_start(xst[:, 0:w], xsT[:, cb * 128:cb * 128 + w])
                xdt = xd_p.tile([128, CMAX_HW * 128], bf16, tag="xdt")
                nc.sync.dma_start(xdt[:, 0:w], xdT[:, cb * 128:cb * 128 + w])
                xsn = xs_p.tile([128, CMAX_HW * 128], bf16, tag="xsn")
                nc.sync.dma_start(xsn[:, 0:w], xsN[:, cb * 128:cb * 128 + w])
                pt = pq_p.tile([128, CMAX_HW, 128], fp32, tag="pt")
                pv = pv_p.tile([128, CMAX_HW, 128], fp32, tag="pv")
                # T' = A^T xd + u (u seeds the accumulation as 2 N=512 rows);
                # logits = T' . xs then covers the bq.k bias term exactly
                half = (cmax // 2) * 128
                nc.tensor.matmul(out=pt[:, 0:cmax // 2, :], lhsT=ones1[:],
                                 rhs=u_tl[:, 0:half], start=True, stop=False,
                                 skip_group_check=True)
                nc.tensor.matmul(out=pt[:, cmax // 2:cmax, :], lhsT=ones1[:],
                                 rhs=u_tl[:, half:cmax * 128], start=True, stop=False,
                                 skip_group_check=True)
                for c in range(cmax):
                    nc.tensor.matmul(out=pt[:, c, :],
                                     lhsT=xdt[:, c * 128:(c + 1) * 128],
                                     rhs=w_a[:], start=False, stop=True,
                                     skip_group_check=True)
                for c in range(cmax):
                    nc.tensor.matmul(out=pv[:, c, :],
                                     lhsT=xst[:, c * 128:(c + 1) * 128],
                                     rhs=w_v[:], start=True, stop=True)
                # logits_e = T'_e . xs_e
                prod = ew_p.tile([128, CMAX_HW, 128], bf16, tag="prod")
                nc.vector.tensor_tensor(
                    out=prod[:, 0:cmax, :], in0=pt[:, 0:cmax, :],
                    in1=xsn[:, 0:w].rearrange("p (c e) -> p c e", e=128),
                    op=Alu.mult)
                lgt = ew_p.tile([128, CMAX_HW], fp32, tag="lgt")
                nc.vector.reduce_sum(out=lgt[:, 0:cmax], in_=prod[:, 0:cmax, :],
                                     axis=mybir.AxisListType.X)
                ex = ew_p.tile([128, CMAX_HW], fp32, tag="ex")
                nc.scalar.activation(ex[:, 0:cmax], lgt[:, 0:cmax], Act.Exp)
                # one-hot oh[e, c, d] streamed from DRAM (host-built)
                oh = oh_p.tile([128, CMAX_HW * 128], bf16, tag="oh")
                nc.sync.dma_start(oh[:, 0:w], oh_d[:, cb * 128:cb * 128 + w])
                # Vex = [v * ex | ex], split DVE / GPSIMD
                vex = ew_p.tile([128, CMAX_HW, 132], bf16, tag="vex")
                ex_b = ex[:, 0:cmax, None].broadcast_to([128, cmax, 128])
                nc.vector.tensor_tensor(out=vex[:, 0:cmax, 0:128],
                                        in0=pv[:, 0:cmax, :], in1=ex_b, op=Alu.mult)
                nc.scalar.activation(vex[:, 0:cmax, 128:129], ex[:, 0:cmax, None],
                                     Act.Copy)
                # scatter: [agg | den] += oh^T @ [Vex | ex]
                pagg = ag_p.tile([128, 512], fp32, tag="pagg")
                for c in range(cmax):
                    nc.tensor.matmul(out=pagg[:, 0:129],
                                     lhsT=oh[:, c * 128:(c + 1) * 128],
                                     rhs=vex[:, c, 0:129],
                                     start=(c == 0), stop=(c == cmax - 1))
                # epilogue: out = elu(agg/den + skip)
                den = epi_p.tile([128, 1], fp32, tag="den")
                nc.vector.tensor_scalar_add(den[:], pagg[:, 128:129], 1e-30)
                rec = epi_p.tile([128, 1], fp32, tag="rec")
                nc.vector.reciprocal(rec[:], den[:])
                z = epi_p.tile([128, 128], fp32, tag="z")
                nc.scalar.activation(z[:], pagg[:, 0:128], Act.Copy, scale=rec[:])
                y = epi_p.tile([128, 128], fp32, tag="y")
                nc.gpsimd.tensor_tensor(out=y[:], in0=z[:], in1=skip_sb[:, b, :],
                                        op=Alu.add)
                e1 = epi_p.tile([128, 128], fp32, tag="e1")
                nc.scalar.activation(e1[:], y[:], Act.Exp)
                p1 = epi_p.tile([128, 128], fp32, tag="p1")
                nc.scalar.activation(p1[:], y[:], Act.Relu)
                ms = epi_p.tile([128, 128], fp32, tag="ms")
                nc.vector.tensor_scalar(out=ms[:], in0=e1[:], scalar1=1.0,
                                        scalar2=-1.0, op0=Alu.min, op1=Alu.add)
                o = epi_p.tile([128, 128], fp32, tag="o")
                nc.gpsimd.tensor_tensor(out=o[:], in0=ms[:], in1=p1[:], op=Alu.add)
                nc.sync.dma_start(out_d[b * 128:(b + 1) * 128, :], o[:])

    nc.compile()
    return nc


_NC_CACHE = {}


def _get_nc(cc):
    if cc not in _NC_CACHE:
        _NC_CACHE[cc] = _build_nc(cc)
    return _NC_CACHE[cc]


def _make_in_maps(inputs, srcn, dstn, dstloc, perm):
    x = np.asarray(inputs["x"], np.float32)
    xb = x.astype(BF16)
    # row 0 sacrificed for pads would corrupt; use an explicit zero row
    xb_pad = np.vstack([xb, np.zeros((1, 128), BF16)])   # index -1 -> zeros
    wq = np.asarray(inputs["Wq"], np.float32)
    wk = np.asarray(inputs["Wk"], np.float32)
    bq = np.asarray(inputs["bq"], np.float32)
    # logits = xd^T A xs + u.xs  with A = s Wq Wk^T, u = s Wk bq
    amat = (SCALE * (wq @ wk.T)).astype(BF16)
    utld = np.tile((SCALE * (wk @ bq)).astype(BF16), CMAX_HW).reshape(1, -1)
    wv = np.asarray(inputs["Wv"], np.float32).astype(BF16)
    ws = np.asarray(inputs["Ws"], np.float32).astype(BF16)
    bsv1 = (np.asarray(inputs["bs"], np.float32)
            + np.asarray(inputs["bv"], np.float32)).astype(BF16).reshape(1, 128)

    S = srcn.shape[2]
    dst_pad = perm.shape[1]
    in_maps = []
    for c in range(M_CORES):
        # column (s*128+e) of xsT = x[src(e, s)]  ->  gather then transpose
        src_lin = srcn[c].T.reshape(-1)          # [S*128] in (s, e) order
        dst_lin = dstn[c].T.reshape(-1)
        xsT = np.ascontiguousarray(xb_pad[src_lin].T)     # [128, S*128]
        xdT = np.ascontiguousarray(xb_pad[dst_lin].T)
        # xsN[e, s*128:(s+1)*128] = x[src(e, s)] row (edge-major, untransposed)
        xsN = xb_pad[srcn[c]].transpose(0, 1, 2).reshape(128, S * 128)
        xs_local = np.zeros((dst_pad, 128), BF16)
        xs_local[:DPC] = xb[c * DPC:(c + 1) * DPC]
        xTs = xs_local[np.minimum(perm[c], dst_pad - 1)].T.copy()
        # one-hot stream: ohs[e, s*128 + d] = (d == dstloc[e, s])
        ohs = np.zeros((128, S, 128), BF16)
        ep, sp = np.nonzero(dstloc[c] != 255.0)
        ohs[ep, sp, dstloc[c][ep, sp].astype(np.int64)] = 1.0
        in_maps.append({
            "xsT": xsT, "xdT": xdT, "xsN": xsN, "xTs": xTs,
            "Amat": amat, "utld": utld, "Wv": wv, "Ws": ws,
            "bsv1": bsv1,
            "ohs": ohs.reshape(128, S * 128),
        })
    return in_maps


def kernel(x, edge_index, Wq, bq, Wk, bk, Wv, bv, Ws, bs):
    from concourse import bass_utils

    srcn, dstn, dstloc, perm, cc = _host_prep(edge_index)
    in_maps = _make_in_maps(
        {"x": x, "Wq": Wq, "Wk": Wk, "Wv": Wv, "Ws": Ws,
         "bq": bq, "bs": bs, "bv": bv}, srcn, dstn, dstloc, perm)
    nc = _get_nc(cc)
    res = bass_utils.run_bass_kernel_spmd(nc, in_maps, core_ids=list(range(M_CORES)))
    out = np.zeros((N, 128), np.float32)
    for c in range(M_CORES):
        rows = res.results[c]["out"]          # [DST_PAD, 128] in device order
        p = perm[c]
        valid = p < DPC
        out[c * DPC + p[valid]] = rows[valid]
    return out
